# revision 17
# baseline (speedup 1.0000x reference)
"""BiLSTM Trainium2 kernel (v3f: transposed-state scan, bf16 matmul operands).

Transposed formulation: gates are computed as g.T[4H, 8] = sum_k Wh_blk.T @
h.T with the 8-wide fused state ([fwd BL | bwd BL]) as the moving operand, so
per step the PE does 64 small matmuls (128-wide bf16 stationary loads
dominate, ~1.5us/step total) instead of streaming the 2048 gate columns per
direction. All elementwise work happens in the [128(H), 4, 8] transposed
layout (free size 32 vs 512 in the batch-major layout), and the h history
stays in SBUF for the output projection.

The scan step is latency-bound (each cross-engine dependency hop costs
hundreds of ns), so the chain is kept to 4 hops: xp is injected into PSUM
through a bf16 identity matmul for ALL gates (no DVE add hop; every sigma
reads PSUM directly); ch's weights are pre-doubled so tanh(x) = 2*sig(2x)-1
lets ONE sigmoid op cover the contiguous f|i|ch psum regions (the 2s-1 fixup
is a fused scalar_tensor_tensor in the ic computation), and the c-chain races
the o-gate's matmul stream so the exposed tail is just sigma(o) -> h.

Phases: (1) transposed xproj -> DRAM [128, 16, T, BL] bf16; (2) fused scan,
bwd xp block-DMA'd time-reversed via negative-stride APs; (3) output
projection reading SBUF-resident hist, fwd+bwd+bias fused on device.
"""

import sys

sys.path.insert(0, "/opt/trn_rl_repo")

import numpy as np
from contextlib import ExitStack

from concourse import bass, bacc, tile, mybir

F32 = mybir.dt.float32
F32R = mybir.dt.float32r
BF16 = mybir.dt.bfloat16
AF = mybir.ActivationFunctionType

B, T, I, H, O = 32, 512, 256, 512, 128
G = 4 * H          # 2048 gate rows, blocks [f | i | o | ch]
BL = B // 8        # 4 batch rows per core
BW = 2 * BL        # 8 state columns: [fwd | bwd]
NCORES = 8
# gate m-slice starts in execution order: ch, i, f, o (the c-chain inputs
# finish early; o - needed only for the final h - streams last). ch's weights
# and bias are pre-doubled on the host so tanh(x) = 2*sigmoid(2x) - 1 lets
# one sigmoid op cover the contiguous f|i|ch psum regions (m-slices 0-11).
GATE_M0 = (8, 4, 0, 12)
M0_F, M0_I, M0_CH, M0_O = 0, 4, 8, 12


def _r(ap):
    return ap.bitcast(F32R)


def _bf16np():
    import ml_dtypes
    return ml_dtypes.bfloat16


def build_program(n_steps=T, repeats=1):
    """Build the per-core Bass program (identical across cores)."""
    assert n_steps % 128 == 0

    nc = bacc.Bacc(
        "TRN2",
        target_bir_lowering=False,
        debug=False,
        num_devices=NCORES,
    )

    rows = n_steps * BL
    xt = nc.dram_tensor("xt", [I, rows], BF16, kind="ExternalInput").ap()
    wxT = nc.dram_tensor("wxT", [I, G], BF16, kind="ExternalInput").ap()
    bxT = nc.dram_tensor("bxT", [G, 1], F32, kind="ExternalInput").ap()
    whT = nc.dram_tensor("whT", [H, G], BF16, kind="ExternalInput").ap()
    h0T = nc.dram_tensor("h0T", [H, BL], F32, kind="ExternalInput").ap()
    c0T = nc.dram_tensor("c0T", [H, BL], F32, kind="ExternalInput").ap()
    wdTf = nc.dram_tensor("wdTf", [H, O], BF16, kind="ExternalInput").ap()
    wdTb = nc.dram_tensor("wdTb", [H, O], BF16, kind="ExternalInput").ap()
    ob = nc.dram_tensor("ob", [O, 1], F32, kind="ExternalInput").ap()
    ident = nc.dram_tensor("ident", [128, 128], BF16, kind="ExternalInput").ap()
    outT = nc.dram_tensor("outT", [O, rows], F32, kind="ExternalOutput").ap()

    xpT_d = nc.dram_tensor("xpT_d", [128, 16, n_steps, BL], BF16, kind="Internal").ap()

    with tile.TileContext(nc) as tc, ExitStack() as ctx:
        const = ctx.enter_context(tc.tile_pool(name="const", bufs=1))
        bigps = ctx.enter_context(tc.tile_pool(name="bigps", bufs=3, space="PSUM"))
        gps = ctx.enter_context(tc.tile_pool(name="gps", bufs=3, space="PSUM"))
        xp_pool = ctx.enter_context(tc.tile_pool(name="xp", bufs=4))
        g_pool = ctx.enter_context(tc.tile_pool(name="g", bufs=8))
        act_pool = ctx.enter_context(tc.tile_pool(name="act", bufs=8))
        tmp_pool = ctx.enter_context(tc.tile_pool(name="tmp", bufs=6))
        hs_pool = ctx.enter_context(tc.tile_pool(name="hs", bufs=3))
        osb_pool = ctx.enter_context(tc.tile_pool(name="osb", bufs=3))

        # ---- constants ----
        xsb = const.tile([128, 2, rows], BF16)
        for c in range(2):
            nc.sync.dma_start(xsb[:, c, :], xt[c * 128:(c + 1) * 128, :])
        wxT_sb = const.tile([128, 2, G], BF16)
        for c in range(2):
            nc.sync.dma_start(wxT_sb[:, c, :], wxT[c * 128:(c + 1) * 128, :])
        whT_sb = const.tile([128, 4, G], BF16)
        for c in range(4):
            nc.sync.dma_start(whT_sb[:, c, :], whT[c * 128:(c + 1) * 128, :])
        bxT_sb = const.tile([128, 16], F32)
        for m in range(16):
            nc.sync.dma_start(bxT_sb[:, m:m + 1], bxT[m * 128:(m + 1) * 128, :])
        id_sb = const.tile([128, 128], BF16)
        nc.sync.dma_start(id_sb[:], ident[:])
        wdT_sb = {}
        for d, src in (("f", wdTf), ("b", wdTb)):
            wdT_sb[d] = const.tile([128, 4, O], BF16, name=f"wdT{d}_sb")
            for c in range(4):
                nc.sync.dma_start(wdT_sb[d][:, c, :], src[c * 128:(c + 1) * 128, :])
        ob_sb = const.tile([O, 1], F32)
        nc.sync.dma_start(ob_sb[:], ob[:])
        zb = const.tile([128, 1], F32)
        nc.gpsimd.memset(zb[:], 0.0)

        # fused scan init state [zeros(fwd) | learned(bwd)]
        z4 = const.tile([128, 4, BW], F32)
        nc.gpsimd.memset(z4[:], 0.0)
        for k in range(4):
            nc.sync.dma_start(z4[:, k, BL:BW], h0T[k * 128:(k + 1) * 128, :])
        h0TF = const.tile([128, 4, BW], BF16)
        nc.vector.tensor_copy(h0TF[:], z4[:])
        cF = const.tile([128, 4, BW], F32)

        # SBUF-resident hidden history, time-aligned per direction
        histf = const.tile([128, 4, n_steps, BL], BF16, name="histf")
        histb = const.tile([128, 4, n_steps, BL], BF16, name="histb")

        for _rep in range(repeats):
            _phases(
                nc, n_steps, xsb, wxT_sb, whT_sb, bxT_sb, id_sb, wdT_sb,
                ob_sb, zb, h0TF, cF, c0T, histf, histb, xpT_d, outT,
                bigps, gps, xp_pool, g_pool, act_pool, tmp_pool, hs_pool,
                osb_pool,
            )

    nc.compile()
    return nc


def _phases(
    nc, n_steps, xsb, wxT_sb, whT_sb, bxT_sb, id_sb, wdT_sb,
    ob_sb, zb, h0TF, cF, c0T, histf, histb, xpT_d, outT,
    bigps, gps, xp_pool, g_pool, act_pool, tmp_pool, hs_pool, osb_pool,
):
    nblk = n_steps // 16
    rows = n_steps * BL
    ncb = rows // 512

    # per-repeat cell-state init (fwd zero, bwd learned)
    nc.gpsimd.memset(cF[:], 0.0)
    for k in range(4):
        nc.sync.dma_start(cF[:, k, BL:BW], c0T[k * 128:(k + 1) * 128, :])

    # ---- phase 1: transposed xproj ----
    for cb in range(ncb):
        for m in range(16):
            ps = bigps.tile([128, 512], F32, tag="bps", name=f"xps{cb}_{m}")
            for c in range(2):
                nc.tensor.matmul(
                    ps[:],
                    wxT_sb[:, c, m * 128:(m + 1) * 128],
                    xsb[:, c, cb * 512:(cb + 1) * 512],
                    start=(c == 0),
                    stop=(c == 1),
                )
            xq = osb_pool.tile([128, 512], BF16, tag="osb", name=f"xq{cb}_{m}")
            nc.scalar.activation(xq[:], ps[:], AF.Identity, bias=bxT_sb[:, m:m + 1])
            nc.sync.dma_start(xpT_d[:, m, cb * 128:(cb + 1) * 128, :], xq[:])

    # ---- phase 2: fused transposed scan ----
    hprev = h0TF
    for blk in range(nblk):
        t0 = blk * 16
        # one tile holds both directions: columns [fwd BL | bwd BL]; the bwd
        # half is DMA'd time-reversed so step tt reads one contiguous [128, BW]
        xpc = xp_pool.tile([128, 16, 16, BW], BF16, tag="xpc", name=f"xpc{blk}")
        nc.sync.dma_start(xpc[:, :, :, 0:BL], xpT_d[:, :, t0:t0 + 16, :])
        rstop = n_steps - 17 - t0
        nc.sync.dma_start(
            xpc[:, :, :, BL:BW],
            xpT_d[:, :, n_steps - 1 - t0:(rstop if rstop >= 0 else None):-1, :],
        )
        for tt in range(16):
            t = t0 + tt
            # one full PSUM bank per step; the very first matmul starts the
            # zero region, the last wh matmul stops it
            ps = gps.tile([128, 512], F32, tag="g", name=f"ps{t}")
            # xp injection for ALL gates (no h dependency: runs during the
            # previous step's tail; every sigma then reads PSUM directly and
            # the DVE add hop disappears from the chain)
            for mi, m in enumerate(range(16)):
                nc.tensor.matmul(
                    ps[:, m * BW:(m + 1) * BW], id_sb[:],
                    xpc[:, m, tt, :],
                    start=(mi == 0), stop=False,
                )
            # recurrent part: moving operand is the 8-wide state
            for m0 in GATE_M0:
                for mm in range(4):
                    m = m0 + mm
                    for k in range(4):
                        nc.tensor.matmul(
                            ps[:, m * BW:(m + 1) * BW],
                            whT_sb[:, k, m * 128:(m + 1) * 128],
                            hprev[:, k, :],
                            start=False,
                            stop=(m0 == GATE_M0[-1] and mm == 3 and k == 3),
                        )

            def ps_gate(m0):
                return ps[:, m0 * BW:(m0 + 4) * BW].rearrange(
                    "p (m w) -> p m w", w=BW
                )

            # one sigmoid covers f|i|ch straight from PSUM (ch pre-doubled:
            # tanh(x) = 2*sigmoid(2x) - 1, fixed up in the ic computation)
            afic = act_pool.tile([128, 12, BW], F32, tag="afic", name=f"afic{t}")
            nc.scalar.activation(
                afic[:],
                ps[:, 0:12 * BW].rearrange("p (m w) -> p m w", w=BW),
                AF.Sigmoid, bias=zb[:, 0:1],
            )
            ao = act_pool.tile([128, 4, BW], F32, tag="ao", name=f"ao{t}")
            nc.scalar.activation(ao[:], ps_gate(M0_O), AF.Sigmoid, bias=zb[:, 0:1])
            # cell update: ic = i * (2*s_ch - 1) = 2*(i*s_ch) - i
            t2 = tmp_pool.tile([128, 4, BW], F32, tag="t2", name=f"t2{t}")
            nc.vector.tensor_mul(t2[:], afic[:, 4:8, :], afic[:, 8:12, :])
            ic = tmp_pool.tile([128, 4, BW], F32, tag="ic", name=f"ic{t}")
            nc.vector.scalar_tensor_tensor(
                ic[:], t2[:], 2.0, afic[:, 4:8, :],
                mybir.AluOpType.mult, mybir.AluOpType.subtract,
            )
            cm = tmp_pool.tile([128, 4, BW], F32, tag="cm", name=f"cm{t}")
            nc.vector.tensor_mul(cm[:], afic[:, 0:4, :], cF[:])
            nc.vector.tensor_add(cF[:], cm[:], ic[:])
            tc2 = tmp_pool.tile([128, 4, BW], F32, tag="tc", name=f"tc{t}")
            nc.scalar.activation(tc2[:], cF[:], AF.Tanh, bias=zb[:, 0:1])
            h = hs_pool.tile([128, 4, BW], BF16, tag="h", name=f"h{t}")
            nc.vector.tensor_mul(h[:], ao[:], tc2[:])
            nc.vector.tensor_copy(histf[:, :, t, :], h[:, :, 0:BL])
            nc.vector.tensor_copy(histb[:, :, n_steps - 1 - t, :], h[:, :, BL:BW])
            hprev = h

    # ---- phase 3: output projection, fwd+bwd+bias fused ----
    for cb in range(ncb):
        ps = bigps.tile([128, 512], F32, tag="bps", name=f"ops{cb}")
        step = 0
        for d, hist in (("f", histf), ("b", histb)):
            for k in range(4):
                rhs = hist[:, k].rearrange("p t b -> p (t b)")[
                    :, cb * 512:(cb + 1) * 512
                ]
                nc.tensor.matmul(
                    ps[:], wdT_sb[d][:, k, :], rhs,
                    start=(step == 0), stop=(step == 7),
                )
                step += 1
        osb = osb_pool.tile([128, 512], F32, tag="osb", name=f"osb{cb}")
        nc.scalar.activation(osb[:], ps[:], AF.Identity, bias=ob_sb[:, 0:1])
        nc.sync.dma_start(outT[:, cb * 512:(cb + 1) * 512], osb[:])


def host_prepare(inputs, n_steps=T):
    """Build the 8 per-core input maps (identical weights, per-core x slice)."""
    bf16 = _bf16np()
    x = np.asarray(inputs["x"], np.float32)
    W = np.concatenate(
        [inputs["Wf_w"], inputs["Wi_w"],
         2.0 * np.asarray(inputs["Wc_w"]), inputs["Wo_w"]], axis=0
    ).astype(np.float32)
    b = np.concatenate(
        [inputs["Wf_b"], inputs["Wi_b"],
         2.0 * np.asarray(inputs["Wc_b"]), inputs["Wo_b"]]
    ).astype(np.float32)
    out_w = np.asarray(inputs["out_w"], np.float32)
    out_b = np.asarray(inputs["out_b"], np.float32)
    bh0 = np.asarray(inputs["bh0"], np.float32).reshape(H, 1)
    bc0 = np.asarray(inputs["bc0"], np.float32).reshape(H, 1)

    shared = {
        "wxT": np.ascontiguousarray(W[:, :I].T).astype(bf16),
        "bxT": b.reshape(G, 1),
        "whT": np.ascontiguousarray(W[:, I:].T).astype(bf16),
        "h0T": np.ascontiguousarray(np.repeat(bh0, BL, axis=1)),
        "c0T": np.ascontiguousarray(np.repeat(bc0, BL, axis=1)),
        "wdTf": np.ascontiguousarray(out_w[:, :H].T).astype(bf16),
        "wdTb": np.ascontiguousarray(out_w[:, H:].T).astype(bf16),
        "ob": out_b.reshape(O, 1),
        "ident": np.eye(128, dtype=np.float32).astype(bf16),
    }
    in_maps = []
    for core in range(NCORES):
        xc = x[core * BL:(core + 1) * BL, :n_steps]          # [BL, T, I]
        xtc = np.ascontiguousarray(
            xc.transpose(2, 1, 0).reshape(I, n_steps * BL)   # col = t*BL + b
        ).astype(bf16)
        in_maps.append({"xt": xtc, **shared})
    return in_maps


def host_gather(results, n_steps=T):
    """Combine per-core outT into [B, T, O]."""
    out = np.empty((B, n_steps, O), np.float32)
    for core in range(NCORES):
        a = results[core]["outT"].reshape(O, n_steps, BL)
        out[core * BL:(core + 1) * BL] = a.transpose(2, 1, 0)
    return out


def _make_runner(nc, n_cores=NCORES):
    """Build a persistent jitted dispatch fn (one trace, reused across calls)."""
    import jax
    from jax.sharding import Mesh, PartitionSpec, NamedSharding
    from jax.experimental.shard_map import shard_map
    from concourse import bass2jax, mybir as _mb

    bass2jax.install_neuronx_cc_hook()

    part_name = nc.partition_id_tensor.name if nc.partition_id_tensor else None
    in_names, out_names, out_avals, zero_outs = [], [], [], []
    for alloc in nc.m.functions[0].allocations:
        if not isinstance(alloc, _mb.MemoryLocationSet):
            continue
        name = alloc.memorylocations[0].name
        if alloc.kind == "ExternalInput":
            if name != part_name:
                in_names.append(name)
        elif alloc.kind == "ExternalOutput":
            out_names.append(name)
            shape = tuple(alloc.tensor_shape)
            dtype = _mb.dt.np(alloc.dtype)
            out_avals.append(jax.core.ShapedArray(shape, dtype))
            zero_outs.append(np.zeros(shape, dtype))
    n_params = len(in_names)
    all_names = list(in_names) + out_names
    if part_name is not None:
        all_names = all_names + [part_name]

    def _body(*args):
        operands = list(args)
        if part_name is not None:
            operands.append(bass2jax.partition_id_tensor())
        outs = bass2jax._bass_exec_p.bind(
            *operands,
            out_avals=tuple(out_avals),
            in_names=tuple(all_names),
            out_names=tuple(out_names),
            lowering_input_output_aliases=(),
            sim_require_finite=True,
            sim_require_nnan=True,
            nc=nc,
        )
        return tuple(outs)

    devices = jax.devices()[:n_cores]
    mesh = Mesh(np.asarray(devices), ("core",))
    spec = PartitionSpec("core")
    nin = n_params + len(out_names)
    fn = jax.jit(
        shard_map(
            _body,
            mesh=mesh,
            in_specs=(spec,) * nin,
            out_specs=(spec,) * len(out_names),
            check_rep=False,
        ),
        keep_unused=True,
    )
    sharding = NamedSharding(mesh, spec)
    return {
        "fn": fn,
        "in_names": in_names,
        "out_names": out_names,
        "out_avals": out_avals,
        "zero_outs": zero_outs,
        "sharding": sharding,
        "n_cores": n_cores,
    }


def _run_spmd(runner, in_maps):
    import jax

    n_cores = runner["n_cores"]
    concat_in = [
        np.concatenate([np.asarray(in_maps[c][nm]) for c in range(n_cores)], axis=0)
        for nm in runner["in_names"]
    ] + [
        np.zeros((n_cores * z.shape[0], *z.shape[1:]), z.dtype)
        for z in runner["zero_outs"]
    ]
    dev_in = [jax.device_put(a, runner["sharding"]) for a in concat_in]
    out = jax.block_until_ready(runner["fn"](*dev_in))
    return [
        {
            nm: np.asarray(out[i]).reshape(n_cores, *runner["out_avals"][i].shape)[c]
            for i, nm in enumerate(runner["out_names"])
        }
        for c in range(n_cores)
    ]


_CACHE = {}


def kernel(**inputs):
    if "runner" not in _CACHE:
        nc = build_program(T)
        _CACHE["nc"] = nc
        _CACHE["runner"] = _make_runner(nc)
    in_maps = host_prepare(inputs, T)
    results = _run_spmd(_CACHE["runner"], in_maps)
    return host_gather(results, T)


# revision 20
# speedup vs baseline: 1.0663x; 1.0663x over previous
"""BiLSTM Trainium2 kernel (v3f: transposed-state scan, bf16 matmul operands).

Transposed formulation: gates are computed as g.T[4H, 8] = sum_k Wh_blk.T @
h.T with the 8-wide fused state ([fwd BL | bwd BL]) as the moving operand, so
per step the PE does 64 small matmuls (128-wide bf16 stationary loads
dominate, ~1.5us/step total) instead of streaming the 2048 gate columns per
direction. All elementwise work happens in the [128(H), 4, 8] transposed
layout (free size 32 vs 512 in the batch-major layout), and the h history
stays in SBUF for the output projection.

The scan step is latency-bound (each cross-engine dependency hop costs
hundreds of ns), so the chain is kept to 4 hops: xp is injected into PSUM
through a bf16 identity matmul for ALL gates (no DVE add hop; every sigma
reads PSUM directly); ch's weights are pre-doubled so tanh(x) = 2*sig(2x)-1
lets ONE sigmoid op cover the contiguous f|i|ch psum regions (the 2s-1 fixup
is a fused scalar_tensor_tensor in the ic computation), and the c-chain races
the o-gate's matmul stream so the exposed tail is just sigma(o) -> h.

Phases: (1) transposed xproj -> DRAM [128, 16, T, BL] bf16; (2) fused scan,
bwd xp block-DMA'd time-reversed via negative-stride APs; (3) output
projection reading SBUF-resident hist, fwd+bwd+bias fused on device.
"""

import sys

sys.path.insert(0, "/opt/trn_rl_repo")

import numpy as np
from contextlib import ExitStack

from concourse import bass, bacc, tile, mybir

F32 = mybir.dt.float32
F32R = mybir.dt.float32r
BF16 = mybir.dt.bfloat16
AF = mybir.ActivationFunctionType

B, T, I, H, O = 32, 512, 256, 512, 128
G = 4 * H          # 2048 gate rows, blocks [f | i | o | ch]
BL = B // 8        # 4 batch rows per core
BW = 2 * BL        # 8 state columns: [fwd | bwd]
NCORES = 8
# gate m-slice starts in execution order: ch, i, f, o (the c-chain inputs
# finish early; o - needed only for the final h - streams last). ch's weights
# and bias are pre-doubled on the host so tanh(x) = 2*sigmoid(2x) - 1 lets
# one sigmoid op cover the contiguous f|i|ch psum regions (m-slices 0-11).
GATE_M0 = (8, 4, 0, 12)
M0_F, M0_I, M0_CH, M0_O = 0, 4, 8, 12


def _r(ap):
    return ap.bitcast(F32R)


def _bf16np():
    import ml_dtypes
    return ml_dtypes.bfloat16


def build_program(n_steps=T, repeats=1):
    """Build the per-core Bass program (identical across cores)."""
    assert n_steps % 128 == 0

    nc = bacc.Bacc(
        "TRN2",
        target_bir_lowering=False,
        debug=False,
        num_devices=NCORES,
    )

    rows = n_steps * BL
    xt = nc.dram_tensor("xt", [I, rows], BF16, kind="ExternalInput").ap()
    wxT = nc.dram_tensor("wxT", [I, G], BF16, kind="ExternalInput").ap()
    bxT = nc.dram_tensor("bxT", [G, 1], F32, kind="ExternalInput").ap()
    whT = nc.dram_tensor("whT", [H, G], BF16, kind="ExternalInput").ap()
    h0T = nc.dram_tensor("h0T", [H, BL], F32, kind="ExternalInput").ap()
    c0T = nc.dram_tensor("c0T", [H, BL], F32, kind="ExternalInput").ap()
    wdTf = nc.dram_tensor("wdTf", [H, O], BF16, kind="ExternalInput").ap()
    wdTb = nc.dram_tensor("wdTb", [H, O], BF16, kind="ExternalInput").ap()
    ob = nc.dram_tensor("ob", [O, 1], F32, kind="ExternalInput").ap()
    ident = nc.dram_tensor("ident", [128, 128], BF16, kind="ExternalInput").ap()
    outT = nc.dram_tensor("outT", [O, rows], F32, kind="ExternalOutput").ap()

    xpT_d = nc.dram_tensor("xpT_d", [128, 16, n_steps, BL], BF16, kind="Internal").ap()

    with tile.TileContext(nc) as tc, ExitStack() as ctx:
        const = ctx.enter_context(tc.tile_pool(name="const", bufs=1))
        bigps = ctx.enter_context(tc.tile_pool(name="bigps", bufs=3, space="PSUM"))
        gps = ctx.enter_context(tc.tile_pool(name="gps", bufs=3, space="PSUM"))
        xp_pool = ctx.enter_context(tc.tile_pool(name="xp", bufs=4))
        g_pool = ctx.enter_context(tc.tile_pool(name="g", bufs=8))
        act_pool = ctx.enter_context(tc.tile_pool(name="act", bufs=8))
        tmp_pool = ctx.enter_context(tc.tile_pool(name="tmp", bufs=6))
        hs_pool = ctx.enter_context(tc.tile_pool(name="hs", bufs=3))
        osb_pool = ctx.enter_context(tc.tile_pool(name="osb", bufs=3))

        # ---- constants ----
        xsb = const.tile([128, 2, rows], BF16)
        for c in range(2):
            nc.sync.dma_start(xsb[:, c, :], xt[c * 128:(c + 1) * 128, :])
        wxT_sb = const.tile([128, 2, G], BF16)
        for c in range(2):
            nc.sync.dma_start(wxT_sb[:, c, :], wxT[c * 128:(c + 1) * 128, :])
        whT_sb = const.tile([128, 4, G], BF16)
        for c in range(4):
            nc.sync.dma_start(whT_sb[:, c, :], whT[c * 128:(c + 1) * 128, :])
        bxT_sb = const.tile([128, 16], F32)
        for m in range(16):
            nc.sync.dma_start(bxT_sb[:, m:m + 1], bxT[m * 128:(m + 1) * 128, :])
        id_sb = const.tile([128, 128], BF16)
        nc.sync.dma_start(id_sb[:], ident[:])
        wdT_sb = {}
        for d, src in (("f", wdTf), ("b", wdTb)):
            wdT_sb[d] = const.tile([128, 4, O], BF16, name=f"wdT{d}_sb")
            for c in range(4):
                nc.sync.dma_start(wdT_sb[d][:, c, :], src[c * 128:(c + 1) * 128, :])
        ob_sb = const.tile([O, 1], F32)
        nc.sync.dma_start(ob_sb[:], ob[:])
        zb = const.tile([128, 1], F32)
        nc.gpsimd.memset(zb[:], 0.0)

        # fused scan init state [zeros(fwd) | learned(bwd)]
        z4 = const.tile([128, 4, BW], F32)
        nc.gpsimd.memset(z4[:], 0.0)
        for k in range(4):
            nc.sync.dma_start(z4[:, k, BL:BW], h0T[k * 128:(k + 1) * 128, :])
        h0TF = const.tile([128, 4, BW], BF16)
        nc.vector.tensor_copy(h0TF[:], z4[:])
        cF = const.tile([128, 4, BW], F32)

        # SBUF-resident hidden history, time-aligned per direction
        histf = const.tile([128, 4, n_steps, BL], BF16, name="histf")
        histb = const.tile([128, 4, n_steps, BL], BF16, name="histb")

        for _rep in range(repeats):
            _phases(
                nc, n_steps, xsb, wxT_sb, whT_sb, bxT_sb, id_sb, wdT_sb,
                ob_sb, zb, h0TF, cF, c0T, histf, histb, xpT_d, outT,
                bigps, gps, xp_pool, g_pool, act_pool, tmp_pool, hs_pool,
                osb_pool,
            )

    nc.compile()
    return nc


def _phases(
    nc, n_steps, xsb, wxT_sb, whT_sb, bxT_sb, id_sb, wdT_sb,
    ob_sb, zb, h0TF, cF, c0T, histf, histb, xpT_d, outT,
    bigps, gps, xp_pool, g_pool, act_pool, tmp_pool, hs_pool, osb_pool,
):
    nblk = n_steps // 16
    rows = n_steps * BL
    ncb = rows // 512

    # per-repeat cell-state init (fwd zero, bwd learned)
    nc.gpsimd.memset(cF[:], 0.0)
    for k in range(4):
        nc.sync.dma_start(cF[:, k, BL:BW], c0T[k * 128:(k + 1) * 128, :])

    # ---- phase 1: transposed xproj ----
    for cb in range(ncb):
        for m in range(16):
            ps = bigps.tile([128, 512], F32, tag="bps", name=f"xps{cb}_{m}")
            for c in range(2):
                nc.tensor.matmul(
                    ps[:],
                    wxT_sb[:, c, m * 128:(m + 1) * 128],
                    xsb[:, c, cb * 512:(cb + 1) * 512],
                    start=(c == 0),
                    stop=(c == 1),
                )
            xq = osb_pool.tile([128, 512], BF16, tag="osb", name=f"xq{cb}_{m}")
            nc.scalar.activation(xq[:], ps[:], AF.Identity, bias=bxT_sb[:, m:m + 1])
            nc.sync.dma_start(xpT_d[:, m, cb * 128:(cb + 1) * 128, :], xq[:])

    # ---- phase 2: fused transposed scan ----
    hprev = h0TF
    for blk in range(nblk):
        t0 = blk * 16
        # one tile holds both directions: columns [fwd BL | bwd BL]; the bwd
        # half is DMA'd time-reversed so step tt reads one contiguous [128, BW]
        xpc = xp_pool.tile([128, 16, 16, BW], BF16, tag="xpc", name=f"xpc{blk}")
        nc.sync.dma_start(xpc[:, :, :, 0:BL], xpT_d[:, :, t0:t0 + 16, :])
        rstop = n_steps - 17 - t0
        nc.sync.dma_start(
            xpc[:, :, :, BL:BW],
            xpT_d[:, :, n_steps - 1 - t0:(rstop if rstop >= 0 else None):-1, :],
        )
        for tt in range(16):
            t = t0 + tt
            # one full PSUM bank per step; the very first matmul starts the
            # zero region, the last wh matmul stops it
            ps = gps.tile([128, 512], F32, tag="g", name=f"ps{t}")
            # xp injection for ALL gates (no h dependency: runs during the
            # previous step's tail; every sigma then reads PSUM directly and
            # the DVE add hop disappears from the chain)
            for mi, m in enumerate(range(16)):
                nc.tensor.matmul(
                    ps[:, m * BW:(m + 1) * BW], id_sb[:],
                    xpc[:, m, tt, :],
                    start=(mi == 0), stop=False,
                )
            # recurrent part: moving operand is the 8-wide state
            for m0 in GATE_M0:
                for mm in range(4):
                    m = m0 + mm
                    for k in range(4):
                        nc.tensor.matmul(
                            ps[:, m * BW:(m + 1) * BW],
                            whT_sb[:, k, m * 128:(m + 1) * 128],
                            hprev[:, k, :],
                            start=False,
                            stop=(m0 == GATE_M0[-1] and mm == 3 and k == 3),
                        )

            def ps_gate(m0):
                return ps[:, m0 * BW:(m0 + 4) * BW].rearrange(
                    "p (m w) -> p m w", w=BW
                )

            # sigma(i|ch) first (contiguous m-slices 4-11, psums land earliest
            # in stream order ch,i,f,o; ch pre-doubled: tanh(x)=2*sig(2x)-1,
            # fixed up in the ic computation) so the DVE c-chain starts before
            # f's psum is even ready; sigma(f) separate
            achi = act_pool.tile([128, 8, BW], F32, tag="achi", name=f"achi{t}")
            nc.scalar.activation(
                achi[:],
                ps[:, 4 * BW:12 * BW].rearrange("p (m w) -> p m w", w=BW),
                AF.Sigmoid, bias=zb[:, 0:1],
            )
            af = act_pool.tile([128, 4, BW], F32, tag="af", name=f"af{t}")
            nc.scalar.activation(af[:], ps_gate(M0_F), AF.Sigmoid, bias=zb[:, 0:1])
            ao = act_pool.tile([128, 4, BW], F32, tag="ao", name=f"ao{t}")
            nc.scalar.activation(ao[:], ps_gate(M0_O), AF.Sigmoid, bias=zb[:, 0:1])
            # cell update: ic = i * (2*s_ch - 1) = 2*(i*s_ch) - i
            t2 = tmp_pool.tile([128, 4, BW], F32, tag="t2", name=f"t2{t}")
            nc.vector.tensor_mul(t2[:], achi[:, 0:4, :], achi[:, 4:8, :])
            ic = tmp_pool.tile([128, 4, BW], F32, tag="ic", name=f"ic{t}")
            nc.vector.scalar_tensor_tensor(
                ic[:], t2[:], 2.0, achi[:, 0:4, :],
                mybir.AluOpType.mult, mybir.AluOpType.subtract,
            )
            cm = tmp_pool.tile([128, 4, BW], F32, tag="cm", name=f"cm{t}")
            nc.vector.tensor_mul(cm[:], af[:], cF[:])
            nc.vector.tensor_add(cF[:], cm[:], ic[:])
            tc2 = tmp_pool.tile([128, 4, BW], F32, tag="tc", name=f"tc{t}")
            nc.scalar.activation(tc2[:], cF[:], AF.Tanh, bias=zb[:, 0:1])
            h = hs_pool.tile([128, 4, BW], BF16, tag="h", name=f"h{t}")
            nc.vector.tensor_mul(h[:], ao[:], tc2[:])
            nc.vector.tensor_copy(histf[:, :, t, :], h[:, :, 0:BL])
            nc.vector.tensor_copy(histb[:, :, n_steps - 1 - t, :], h[:, :, BL:BW])
            hprev = h

    # ---- phase 3: output projection, fwd+bwd+bias fused ----
    for cb in range(ncb):
        ps = bigps.tile([128, 512], F32, tag="bps", name=f"ops{cb}")
        step = 0
        for d, hist in (("f", histf), ("b", histb)):
            for k in range(4):
                rhs = hist[:, k].rearrange("p t b -> p (t b)")[
                    :, cb * 512:(cb + 1) * 512
                ]
                nc.tensor.matmul(
                    ps[:], wdT_sb[d][:, k, :], rhs,
                    start=(step == 0), stop=(step == 7),
                )
                step += 1
        osb = osb_pool.tile([128, 512], F32, tag="osb", name=f"osb{cb}")
        nc.scalar.activation(osb[:], ps[:], AF.Identity, bias=ob_sb[:, 0:1])
        nc.sync.dma_start(outT[:, cb * 512:(cb + 1) * 512], osb[:])


def host_prepare(inputs, n_steps=T):
    """Build the 8 per-core input maps (identical weights, per-core x slice)."""
    bf16 = _bf16np()
    x = np.asarray(inputs["x"], np.float32)
    W = np.concatenate(
        [inputs["Wf_w"], inputs["Wi_w"],
         2.0 * np.asarray(inputs["Wc_w"]), inputs["Wo_w"]], axis=0
    ).astype(np.float32)
    b = np.concatenate(
        [inputs["Wf_b"], inputs["Wi_b"],
         2.0 * np.asarray(inputs["Wc_b"]), inputs["Wo_b"]]
    ).astype(np.float32)
    out_w = np.asarray(inputs["out_w"], np.float32)
    out_b = np.asarray(inputs["out_b"], np.float32)
    bh0 = np.asarray(inputs["bh0"], np.float32).reshape(H, 1)
    bc0 = np.asarray(inputs["bc0"], np.float32).reshape(H, 1)

    shared = {
        "wxT": np.ascontiguousarray(W[:, :I].T).astype(bf16),
        "bxT": b.reshape(G, 1),
        "whT": np.ascontiguousarray(W[:, I:].T).astype(bf16),
        "h0T": np.ascontiguousarray(np.repeat(bh0, BL, axis=1)),
        "c0T": np.ascontiguousarray(np.repeat(bc0, BL, axis=1)),
        "wdTf": np.ascontiguousarray(out_w[:, :H].T).astype(bf16),
        "wdTb": np.ascontiguousarray(out_w[:, H:].T).astype(bf16),
        "ob": out_b.reshape(O, 1),
        "ident": np.eye(128, dtype=np.float32).astype(bf16),
    }
    in_maps = []
    for core in range(NCORES):
        xc = x[core * BL:(core + 1) * BL, :n_steps]          # [BL, T, I]
        xtc = np.ascontiguousarray(
            xc.transpose(2, 1, 0).reshape(I, n_steps * BL)   # col = t*BL + b
        ).astype(bf16)
        in_maps.append({"xt": xtc, **shared})
    return in_maps


def host_gather(results, n_steps=T):
    """Combine per-core outT into [B, T, O]."""
    out = np.empty((B, n_steps, O), np.float32)
    for core in range(NCORES):
        a = results[core]["outT"].reshape(O, n_steps, BL)
        out[core * BL:(core + 1) * BL] = a.transpose(2, 1, 0)
    return out


def _make_runner(nc, n_cores=NCORES):
    """Build a persistent jitted dispatch fn (one trace, reused across calls)."""
    import jax
    from jax.sharding import Mesh, PartitionSpec, NamedSharding
    from jax.experimental.shard_map import shard_map
    from concourse import bass2jax, mybir as _mb

    bass2jax.install_neuronx_cc_hook()

    part_name = nc.partition_id_tensor.name if nc.partition_id_tensor else None
    in_names, out_names, out_avals, zero_outs = [], [], [], []
    for alloc in nc.m.functions[0].allocations:
        if not isinstance(alloc, _mb.MemoryLocationSet):
            continue
        name = alloc.memorylocations[0].name
        if alloc.kind == "ExternalInput":
            if name != part_name:
                in_names.append(name)
        elif alloc.kind == "ExternalOutput":
            out_names.append(name)
            shape = tuple(alloc.tensor_shape)
            dtype = _mb.dt.np(alloc.dtype)
            out_avals.append(jax.core.ShapedArray(shape, dtype))
            zero_outs.append(np.zeros(shape, dtype))
    n_params = len(in_names)
    all_names = list(in_names) + out_names
    if part_name is not None:
        all_names = all_names + [part_name]

    def _body(*args):
        operands = list(args)
        if part_name is not None:
            operands.append(bass2jax.partition_id_tensor())
        outs = bass2jax._bass_exec_p.bind(
            *operands,
            out_avals=tuple(out_avals),
            in_names=tuple(all_names),
            out_names=tuple(out_names),
            lowering_input_output_aliases=(),
            sim_require_finite=True,
            sim_require_nnan=True,
            nc=nc,
        )
        return tuple(outs)

    devices = jax.devices()[:n_cores]
    mesh = Mesh(np.asarray(devices), ("core",))
    spec = PartitionSpec("core")
    nin = n_params + len(out_names)
    fn = jax.jit(
        shard_map(
            _body,
            mesh=mesh,
            in_specs=(spec,) * nin,
            out_specs=(spec,) * len(out_names),
            check_rep=False,
        ),
        keep_unused=True,
    )
    sharding = NamedSharding(mesh, spec)
    return {
        "fn": fn,
        "in_names": in_names,
        "out_names": out_names,
        "out_avals": out_avals,
        "zero_outs": zero_outs,
        "sharding": sharding,
        "n_cores": n_cores,
    }


def _run_spmd(runner, in_maps):
    import jax

    n_cores = runner["n_cores"]
    concat_in = [
        np.concatenate([np.asarray(in_maps[c][nm]) for c in range(n_cores)], axis=0)
        for nm in runner["in_names"]
    ] + [
        np.zeros((n_cores * z.shape[0], *z.shape[1:]), z.dtype)
        for z in runner["zero_outs"]
    ]
    dev_in = [jax.device_put(a, runner["sharding"]) for a in concat_in]
    out = jax.block_until_ready(runner["fn"](*dev_in))
    return [
        {
            nm: np.asarray(out[i]).reshape(n_cores, *runner["out_avals"][i].shape)[c]
            for i, nm in enumerate(runner["out_names"])
        }
        for c in range(n_cores)
    ]


_CACHE = {}


def kernel(**inputs):
    if "runner" not in _CACHE:
        nc = build_program(T)
        _CACHE["nc"] = nc
        _CACHE["runner"] = _make_runner(nc)
    in_maps = host_prepare(inputs, T)
    results = _run_spmd(_CACHE["runner"], in_maps)
    return host_gather(results, T)
